# revision 55
# baseline (speedup 1.0000x reference)
"""GCNConv-style kernel: cosine-weighted global edge softmax aggregation + linear + residual.

Device strategy (8 NeuronCores, edges sharded by destination-row range):
  - Host: L2-normalize x -> fp16 table; sort edges by dest row; bucket each
    core's edges into 128-row windows with DATA-DRIVEN chunk counts (CH[w] =
    max fill over cores); precompute the pure-0/1 scatter one-hot in fp8.
  - Device (per core): batched indirect-DMA gathers of xn16[row] / xn16[col]
    (256B rows, one merged gather per 7-window group), per-edge cosine dot via
    DVE mult + split reduce (1/4 of chunks on ACT via accum_out, rest one DVE
    3D pass), p = exp(s-1) on ACT (accum_out gives the local softmax
    denominator partials), qoh = oh_fp8 * (p*norm[col]) split the same way
    (ACT per-chunk scale-copy / DVE whole-window broadcast mult), TensorE
    matmul-scatter into PSUM per 128-row window, then out_u^T = W @ aggr_u^T.
  - The selector+scatter stage is SOFTWARE-PIPELINED two windows behind the
    prod/reduce/exp stage so the in-order DVE/ACT streams never stall on each
    other's results (this was worth ~60us of the 320->258us step).
  - The softmax denominator Z factors out of the segment sum entirely; host
    sums the per-core partials (minus the known padding contribution) and
    applies out = out_u / Z + x + b.
Measured: 369us (first working) -> 257-262us; engines balanced at DVE ~180us /
ACT ~190us / DMA ~160us per queue; rel err 1.3e-05.

Execution strategy (three tiers):
  1. Bass NEFF (above) — compiles cleanly, but this container's axon tunnel
     rejects executing ANY bass NEFF (even trivial ones), so a failure marker
     gates it; it re-arms automatically if the environment is fixed.
  2. JAX op-by-op on the NeuronCores — gathers, softmax, segment-sum and the
     linear all run on device as small XLA NEFFs out of the persistent
     compile cache (~1.2s end-to-end; a monolithic jit fails neuronx-cc).
  3. Pure-numpy host fallback (~4s) if JAX dispatch is unavailable.
"""

import os

import numpy as np

N_NODES = 50000
N_EDGES = 600000
D = 128
N_CORES = 8
RPC = N_NODES // N_CORES  # 6250 rows per core
W = 128  # window rows
NW = 49  # windows per core (48 full + one 106-row tail)
TAIL_ROWS = RPC - 48 * W  # 106
# gather groups: smaller groups (+ bufs=3 on the gather pool) give the
# indirect gather ~2 group-times of slack instead of barely one, removing
# the periodic 6-8us group-boundary stalls on DVE/ACT; a small first group
# also cuts the startup wait for the first gather
# one tiny leading group so compute starts after a ~5us gather instead of
# waiting out the full first 7-window gather (~18us); steady state keeps the
# proven 7-window gather granularity (many small groups measured WORSE)
GROUPS = [(0, 2)] + [(2 + 7 * g, min(2 + 7 * (g + 1), 49)) for g in range(7)]

_state = {}


class _Layout:
    """Packed-tensor layout, data-driven: CH[w] = chunks (of 128 edges) for
    window w, sized as max over cores of the actual bucket fill so padding
    tracks the realized edge distribution instead of a worst-case bound."""

    def __init__(self, CH):
        self.CH = list(CH)
        self.WINBASE = np.concatenate(
            [[0], np.cumsum(np.array(self.CH) * 128)]
        ).astype(np.int64)
        self.NCH = int(sum(self.CH))
        self.E_PAD = self.NCH * 128
        self.NCH2 = (self.NCH + 1) // 2  # i32 cols holding f16 rloc
        # packed int32 layout (columns)
        self.OFF_GIDX = 0  # [2*NCH] per-group [row idxs | col idxs]
        self.OFF_RLOC = 2 * self.NCH  # [NCH2] -> f16 bitcast [2*NCH2]
        self.OFF_SCL = self.OFF_RLOC + self.NCH2  # [NCH] fp32 bitcast
        self.OFF_IOTA = self.OFF_SCL + self.NCH  # [64] f16 -> [128,128]
        self.OFF_WT = self.OFF_IOTA + 64  # [128] fp32 -> [128,128]
        self.PK = self.OFF_WT + 128


def _build_bass(L):
    import concourse.bass as bass
    from concourse import mybir, tile

    f16 = mybir.dt.float16
    f32 = mybir.dt.float32
    i32 = mybir.dt.int32
    Alu = mybir.AluOpType

    CH, WINBASE = L.CH, L.WINBASE
    OFF_RLOC, OFF_SCL = L.OFF_RLOC, L.OFF_SCL
    OFF_IOTA, OFF_WT, PK = L.OFF_IOTA, L.OFF_WT, L.PK

    nc = bass.Bass()
    xn16 = nc.dram_tensor("xn16", [N_NODES, D], f16, kind="ExternalInput")
    packed = nc.dram_tensor("packed", [128, PK], i32, kind="ExternalInput")
    # host-precomputed one-hot (pure 0/1 so fp8 is exact — halves its DMA):
    # oh16[p, c*128+r] = (rloc[c,p]==r); norm[col] is folded into q on-device
    f8 = mybir.dt.float8e4
    oh16 = nc.dram_tensor("oh16", [128, L.E_PAD], f8, kind="ExternalInput")
    outall = nc.dram_tensor("outall", [D, RPC + NW], f32, kind="ExternalOutput")

    with tile.TileContext(nc) as tc:
        with (
            tc.tile_pool(name="idx", bufs=1) as idxp,
            tc.tile_pool(name="gath", bufs=2) as gathp,
            tc.tile_pool(name="work", bufs=3) as workp,
            tc.tile_pool(name="small", bufs=3) as smallp,
            tc.tile_pool(name="acc", bufs=1) as accp,
            tc.tile_pool(name="psw", bufs=4, space="PSUM") as pswp,
            tc.tile_pool(name="psf", bufs=2, space="PSUM") as psfp,
            tc.tile_pool(name="psd", bufs=1, space="PSUM") as psdp,
        ):
            pk_t = idxp.tile([128, PK], i32)
            nc.sync.dma_start(pk_t[:], packed[:])
            iota_ap = pk_t[:, OFF_IOTA : OFF_IOTA + 64].bitcast(f16)  # [128,128]
            wt_ap = pk_t[:, OFF_WT : OFF_WT + 128].bitcast(f32)  # [128,128]

            neg1_t = idxp.tile([128, 1], f32)
            nc.vector.memset(neg1_t[:], -1.0)
            # ACT scratch for per-chunk accum-reduce dummy writes
            ascr_t = idxp.tile([128, 128], f16)
            # make DVE observe the packed-DMA semaphore once, so later DVE ops
            # (single sync-wait slot) never need to wait on it
            obs_t = idxp.tile([128, 1], f32)
            nc.vector.tensor_copy(
                out=obs_t[:], in_=pk_t[:, OFF_RLOC : OFF_RLOC + 1].bitcast(f32)
            )

            aggrT = accp.tile([128, RPC], f32)  # [d, local_row]
            zp_t = accp.tile([128, NW], f32)
            outU = accp.tile([128, RPC + NW], f32)
            fin = [
                (t * 512, min(512, RPC - t * 512))
                for t in range((RPC + 511) // 512)
            ]
            fin_done = set()

            def emit_fin(t):
                # out_u^T tile = W @ aggr_u^T tile (lhsT = W^T from packed);
                # emitted as soon as its 4 source windows have drained so the
                # final matmul overlaps the main loop instead of a serial tail
                fin_done.add(t)
                t0, tn = fin[t]
                psf = psfp.tile([128, 512], f32, tag="psf")
                nc.tensor.matmul(
                    psf[:, :tn],
                    lhsT=wt_ap,
                    rhs=aggrT[:, t0 : t0 + tn],
                    start=True,
                    stop=True,
                )
                nc.scalar.activation(
                    out=outU[:, t0 : t0 + tn],
                    in_=psf[:, :tn],
                    func=mybir.ActivationFunctionType.Copy,
                )

            # PE observer: absorb the packed-DMA (HWDGE) semaphore on the PE
            # clock once, so later matmuls reading wt_ap need no extra wait
            dps0 = psdp.tile([1, 1], f32, tag="dps0")
            nc.tensor.matmul(
                dps0[:], lhsT=wt_ap[:, :1], rhs=wt_ap[:, :1], start=True, stop=True
            )

            # software-pipelined by one window: the selector mult + scatter
            # matmuls for window w are emitted AFTER window w+1's prod/reduce,
            # so the in-order DVE stream never stalls waiting for ACT's exp
            # (it runs during the next window's prod+reduce)
            def emit_back(st):
                w, C, cw, xg_b, c_sl_b, oh_b, p_b_t = st
                # q[e,c] = p[e,c] * norm[col[e,c]] (scl from packed)
                q_t = smallp.tile([128, C], f32, tag="q")
                nc.vector.tensor_tensor(
                    out=q_t[:],
                    in0=p_b_t[:],
                    in1=pk_t[:, OFF_SCL + cw : OFF_SCL + cw + C].bitcast(f32),
                    op=Alu.mult,
                )
                # qoh[e,(c,r)] = oh[e,(c,r)] * q[e,c]; chunks [0,ka) go to
                # the otherwise-idle ACT engine (per-chunk scale-copy), the
                # rest stay a single whole-window DVE pass — DVE is the
                # kernel's critical engine
                ka = C // 4 + 1  # one more chunk than the reduce split:
                # DVE measures ~16us busier than ACT, this evens them out
                qoh = workp.tile([128, C * 128], f16, tag="qoh")
                for j in range(ka):
                    nc.scalar.activation(
                        out=qoh[:, j * 128 : (j + 1) * 128],
                        in_=oh_b[:, j * 128 : (j + 1) * 128],
                        func=mybir.ActivationFunctionType.Copy,
                        scale=q_t[:, j : j + 1],
                    )
                q_b = (
                    q_t[:, ka:]
                    .rearrange("p c -> p c ()")
                    .broadcast_to([128, C - ka, 128])
                )
                nc.vector.tensor_tensor(
                    out=qoh[:, ka * 128 :].rearrange("p (c d) -> p c d", d=128),
                    in0=oh_b[:, ka * 128 :].rearrange("p (c d) -> p c d", d=128),
                    in1=q_b,
                    op=Alu.mult,
                )
                ps = pswp.tile([128, 128], f32, tag="psw")
                for j in range(C):
                    nc.tensor.matmul(
                        ps[:],
                        lhsT=xg_b[
                            :,
                            c_sl_b.start + j * 128 : c_sl_b.start + (j + 1) * 128,
                        ],
                        rhs=qoh[:, j * 128 : (j + 1) * 128],
                        start=(j == 0),
                        stop=(j == C - 1),
                    )
                cols = W if w < 48 else TAIL_ROWS
                # PSUM drain on ACT (idle engine) to keep DVE free
                nc.scalar.activation(
                    out=aggrT[:, w * W : w * W + cols],
                    in_=ps[:, :cols],
                    func=mybir.ActivationFunctionType.Copy,
                )

            pending = []
            for g0, g1 in GROUPS:
                c0 = int(WINBASE[g0] // 128)
                c1 = int(WINBASE[g1] // 128)
                ncg = c1 - c0
                xg_t = gathp.tile([128, 2 * ncg * 128], f16, tag="xg")
                nc.gpsimd.indirect_dma_start(
                    out=xg_t[:],
                    out_offset=None,
                    in_=xn16[:],
                    in_offset=bass.IndirectOffsetOnAxis(
                        ap=pk_t[:, 2 * c0 : 2 * c1], axis=0
                    ),
                )
                # PE observer: absorb this gather's DMASW semaphore on the PE
                # clock so the window matmuls reading xg_t keep a single wait
                dps1 = psdp.tile([1, 1], f32, tag="dps1")
                nc.tensor.matmul(
                    dps1[:], lhsT=xg_t[:, :1], rhs=xg_t[:, :1], start=True, stop=True
                )
                for w in range(g0, g1):
                    C = CH[w]
                    cw = int(WINBASE[w] // 128)  # absolute chunk index
                    off = cw - c0  # chunk offset in group
                    r_sl = slice(off * 128, (off + C) * 128)
                    c_sl = slice((ncg + off) * 128, (ncg + off + C) * 128)
                    oh_t = workp.tile([128, C * 128], f8, tag="oh")
                    nc.sync.dma_start(
                        oh_t[:],
                        oh16[:, int(WINBASE[w]) : int(WINBASE[w]) + C * 128],
                    )
                    prod = workp.tile([128, C * 128], f16, tag="prod")
                    nc.vector.tensor_tensor(
                        out=prod[:], in0=xg_t[:, r_sl], in1=xg_t[:, c_sl], op=Alu.mult
                    )
                    s_t = smallp.tile([128, C], f32, tag="s")
                    # dot-reduce split: chunks [0,ka) on ACT (copy w/
                    # accum_out gives the per-chunk sum), rest one DVE pass
                    ka = C // 4
                    for j in range(ka):
                        nc.scalar.activation(
                            out=ascr_t[:],
                            in_=prod[:, j * 128 : (j + 1) * 128],
                            func=mybir.ActivationFunctionType.Copy,
                            accum_out=s_t[:, j : j + 1],
                        )
                    nc.vector.tensor_reduce(
                        out=s_t[:, ka:],
                        in_=prod[:, ka * 128 :].rearrange(
                            "p (c d) -> p c d", d=128
                        ),
                        axis=mybir.AxisListType.X,
                        op=Alu.add,
                    )
                    p_t = smallp.tile([128, C], f32, tag="p")
                    nc.scalar.activation(
                        out=p_t[:],
                        in_=s_t[:],
                        func=mybir.ActivationFunctionType.Exp,
                        bias=neg1_t[:],
                        accum_out=zp_t[:, w : w + 1],
                    )
                    if len(pending) == 2:
                        emit_back(pending.pop(0))
                    pending.append((w, C, cw, xg_t, c_sl, oh_t, p_t))
            for st in pending:
                emit_back(st)

            # stream each output tile to HBM as soon as its copy lands, so
            # the out DMA overlaps the remaining fin matmuls instead of one
            # serial 3.2MB transfer at the very end
            for t in range(len(fin)):
                if t not in fin_done:
                    emit_fin(t)
                t0, tn = fin[t]
                nc.sync.dma_start(
                    outall[:, t0 : t0 + tn], outU[:, t0 : t0 + tn]
                )
            nc.vector.tensor_copy(out=outU[:, RPC : RPC + NW], in_=zp_t[:])
            nc.sync.dma_start(
                outall[:, RPC : RPC + NW], outU[:, RPC : RPC + NW]
            )
    _strip_self_engine_waits(nc)
    _reduce_waits(nc)
    # run the full Bacc legalization pipeline (matmul wait moves, event-sem
    # wait splitting, regalloc) — the axon/bass2jax path does not call it
    nc.finalize()
    return nc


def _reduce_waits(nc):
    """Transitive vector-clock reduction of semaphore waits.

    Walrus codegen only allows one sync-wait per instruction (for most
    instruction structs), but Tile emits one wait per dependency. Engines
    execute their streams in order, so a wait on sem S at value v implies
    every fact the updater of (S, v) had itself observed. This pass walks the
    scheduled program, tracks each engine's provable knowledge, drops implied
    waits, and — when an instruction still holds several waits — replaces
    them with a single dominating wait chosen from earlier-scheduled updates
    (earlier in a valid topological order, hence deadlock-free).
    """
    import bisect

    from concourse import mybir

    ASYNC_PREFIXES = ("DMASW", "DMAHW", "Collectives")
    # Only Tile DATA semaphores have monotonic-counter semantics. Barrier/
    # event semaphores (barrier_*, block_sem, ...) are used by
    # InstEventSemaphore/InstDrain with ==0 / >=N / decrement semantics;
    # modeling them as counters strips their waits, which produces
    # event-barrier encodings that nrt_load REJECTS (the whole-NEFF
    # "<redacted>" load failure). Treat them as opaque: never drop their
    # waits, never count their updates.
    DATA_PREFIXES = (
        "DVE_",
        "Pool_",
        "Activation_",
        "PE_",
        "SP_",
        "DMASW",
        "DMAHW",
        "Collectives",
    )

    def is_data(name):
        return (name or "").startswith(DATA_PREFIXES)

    sem_count = {}
    snap_vals = {}  # sem -> [values]
    snap_vcs = {}  # sem -> [vc dict at that value]
    vc_eng = {}
    sem_ids = {}  # sem ant_name -> numeric id

    def join(a, b):
        for k, v in b.items():
            if a.get(k, 0) < v:
                a[k] = v

    def know_at(S, v):
        vals = snap_vals.get(S)
        if not vals:
            return {}
        i = bisect.bisect_left(vals, v)
        if i >= len(vals):
            i = len(vals) - 1
        return snap_vcs[S][i]

    def dominating_single(kept):
        # Find one of the instruction's OWN waits whose updater's knowledge
        # implies all the others. Restricting candidates to the original
        # waits keeps the dependency graph a subgraph of Tile's (acyclic), so
        # this can never introduce a deadlock.
        need = {w.ant_name: w.wait_value for w in kept}
        for w in kept:
            vc = know_at(w.ant_name, w.wait_value)
            if all(
                vc.get(n, 0) >= v
                for n, v in need.items()
                if n != w.ant_name
            ):
                return w
        return None

    import bass_rust

    nop_n = [0]

    for blk in nc.m.functions[0].blocks:
        new_insts = []
        for inst in blk.instructions:
            si = inst.sync_info
            eng = getattr(inst, "engine", None)
            vc = vc_eng.setdefault(eng, {})
            if si is not None and si.on_wait:
                keep_other = [w for w in si.on_wait if not is_data(w.ant_name)]
                kept = []
                for w in si.on_wait:
                    if not is_data(w.ant_name):
                        continue
                    S, v = w.ant_name, w.wait_value
                    sem_ids.setdefault(S, w.id)
                    if vc.get(S, 0) >= v:
                        continue
                    kept.append(w)
                if len(kept) > 1:
                    dom = dominating_single(kept)
                    if dom is not None:
                        kept = [dom]
                for w in kept:
                    S, v = w.ant_name, w.wait_value
                    join(vc, know_at(S, v))
                    if vc.get(S, 0) < v:
                        vc[S] = v
                if len(kept) + len(keep_other) > 1:
                    # walrus allows one sync-wait per instruction: peel the
                    # extras onto preceding same-engine NOPs (engines run
                    # their streams in order, so this is equivalent)
                    for w in kept[:-1]:
                        nop_n[0] += 1
                        new_insts.append(
                            bass_rust.InstNoOp(
                                name=f"I-waitnop{nop_n[0]}",
                                engine=eng,
                                sync_info=mybir.SyncInfo(
                                    on_wait=[w], on_update=[]
                                ),
                            )
                        )
                    kept = kept[-1:]
                kept = keep_other + kept
                if len(kept) != len(si.on_wait):
                    inst.sync_info = mybir.SyncInfo(
                        on_wait=kept, on_update=si.on_update
                    )
                si = inst.sync_info
            if si is not None:
                for u in si.on_update:
                    S = u.ant_name
                    if not is_data(S):
                        continue
                    sem_ids.setdefault(S, u.id)
                    sem_count[S] = sem_count.get(S, 0) + u.update_value
                    snap = dict(vc)
                    snap[S] = sem_count[S]
                    snap_vals.setdefault(S, []).append(sem_count[S])
                    snap_vcs.setdefault(S, []).append(snap)
                    if not S.startswith(ASYNC_PREFIXES):
                        # synchronous (engine) sem: the issuing engine knows
                        # its own update happened
                        if vc.get(S, 0) < sem_count[S]:
                            vc[S] = sem_count[S]
            new_insts.append(inst)
        blk.instructions = new_insts


def _strip_self_engine_waits(nc):
    """Remove semaphore waits on an instruction's own engine-completion sem.

    Engines execute their instruction streams in order, so a wait on the
    issuing engine's own sem is always already satisfied; walrus codegen
    rejects instructions with more than one sync wait, and Tile emits these
    redundant self-waits for slot-reuse WAW deps.
    """
    from concourse import mybir

    for blk in nc.m.functions[0].blocks:
        for inst in blk.instructions:
            si = inst.sync_info
            if si is None or not si.on_wait:
                continue
            eng = getattr(inst, "engine", None)
            if eng is None:
                continue
            pref = {
                mybir.EngineType.DVE: "DVE_",
                mybir.EngineType.Activation: "Activation_",
                mybir.EngineType.PE: "PE_",
                mybir.EngineType.Pool: "Pool_",
                mybir.EngineType.SP: "SP_",
            }.get(eng)
            if pref is None:
                continue
            kept = [w for w in si.on_wait if not (w.ant_name or "").startswith(pref)]
            if len(kept) != len(si.on_wait):
                inst.sync_info = mybir.SyncInfo(on_wait=kept, on_update=si.on_update)


def _prep_inputs(x, edge_index, W_mat):
    """Host-side edge bucketing; returns per-core input maps + spill info."""
    x = np.ascontiguousarray(x, dtype=np.float32)
    norm = np.maximum(np.sqrt((x * x).sum(axis=1)), 1e-12)
    xn = x / norm[:, None]
    xn16 = xn.astype(np.float16)

    row = np.asarray(edge_index[0], dtype=np.int64)
    col = np.asarray(edge_index[1], dtype=np.int64)
    order = np.argsort(row, kind="stable")
    rs = row[order]
    cs = col[order]
    core = rs // RPC
    lrow = rs - core * RPC
    w = np.minimum(lrow // W, NW - 1)
    gid = core * NW + w
    counts = np.bincount(gid, minlength=N_CORES * NW)
    # data-driven chunk counts: SPMD => one program for all 8 cores, so
    # CH[w] is the max fill over cores (vs a static worst-case bound)
    cw_counts = counts.reshape(N_CORES, NW).max(axis=0)
    CH = np.maximum((cw_counts + 127) // 128, 1).astype(np.int64)
    L = _Layout(CH.tolist())
    WINBASE, NCH, E_PAD = L.WINBASE, L.NCH, L.E_PAD

    first = np.concatenate([[0], np.cumsum(counts)[:-1]])
    # edges sorted by row => grouped by gid in order
    pos = np.arange(len(rs)) - first[gid]
    caps = np.tile(np.array(L.CH) * 128, N_CORES)
    ok = pos < caps[gid]
    spill = ~ok
    dest = WINBASE[w[ok]] + pos[ok]  # slot within core's padded edge list

    colp = np.zeros((N_CORES, E_PAD), dtype=np.int64)
    rowp = np.zeros((N_CORES, E_PAD), dtype=np.int64)
    rlocp = np.full((N_CORES, E_PAD), -5.0, dtype=np.float32)
    sclp = np.zeros((N_CORES, E_PAD), dtype=np.float32)
    ck = core[ok]
    colp[ck, dest] = cs[ok]
    rowp[ck, dest] = rs[ok]
    rlocp[ck, dest] = (lrow[ok] - w[ok] * W).astype(np.float32)
    sclp[ck, dest] = norm[cs[ok]].astype(np.float32)

    def lay(a, dt):  # [E_PAD] -> [128, NCH] with element [p, j] = a[j*128+p]
        return np.ascontiguousarray(a.reshape(NCH, 128).T.astype(dt))

    iota_i = (
        np.broadcast_to(np.arange(128, dtype=np.float16)[None, :], (128, 128))
        .copy()
        .view(np.int32)
    )
    wt_i = np.ascontiguousarray(W_mat.astype(np.float32).T).view(np.int32)

    in_maps = []
    for c in range(N_CORES):
        ri = lay(rowp[c], np.int32)
        ci = lay(colp[c], np.int32)
        pk = np.empty((128, L.PK), dtype=np.int32)
        for g0, g1 in GROUPS:
            c0 = int(WINBASE[g0] // 128)
            c1 = int(WINBASE[g1] // 128)
            ncg = c1 - c0
            pk[:, 2 * c0 : 2 * c0 + ncg] = ri[:, c0:c1]
            pk[:, 2 * c0 + ncg : 2 * c1] = ci[:, c0:c1]
        rl16 = lay(rlocp[c], np.float16)  # [128, NCH] f16
        if NCH % 2:
            rl16 = np.concatenate(
                [rl16, np.zeros((128, 1), np.float16)], axis=1
            )
        pk[:, L.OFF_RLOC : L.OFF_RLOC + L.NCH2] = np.ascontiguousarray(
            rl16
        ).view(np.int32)
        pk[:, L.OFF_SCL : L.OFF_SCL + NCH] = lay(sclp[c], np.float32).view(
            np.int32
        )
        pk[:, L.OFF_IOTA : L.OFF_IOTA + 64] = iota_i
        pk[:, L.OFF_WT : L.OFF_WT + 128] = wt_i
        # 0/1 one-hot [chunk, edge, row] -> [128, NCH*128], fp8 (exact)
        import ml_dtypes

        oh3 = np.zeros((NCH, 128, 128), dtype=np.uint8)
        rl = rlocp[c].reshape(NCH, 128)
        ci_, pi_ = np.nonzero(rl >= 0.0)
        oh3[ci_, pi_, rl[ci_, pi_].astype(np.int64)] = 0x38  # 1.0 in e4m3
        oh = np.ascontiguousarray(
            oh3.transpose(1, 0, 2).reshape(128, NCH * 128)
        ).view(ml_dtypes.float8_e4m3)
        in_maps.append({"xn16": xn16, "packed": pk, "oh16": oh})
    n_pad = N_CORES * E_PAD - int(ok.sum())
    spill_info = None
    if spill.any():
        spill_info = (rs[spill], cs[spill])
    return L, in_maps, xn, norm, n_pad, spill_info


_DEVICE_BROKEN_MARKER = "/tmp/.bass_device_broken"


def _jax_device_path(x, edge_index, W, b):
    """Run the full computation on the NeuronCores via XLA (jax on the axon
    backend). Used when the bass NEFF path is unavailable; the jit is cached
    across processes by the persistent neuron compile cache."""
    # Op-by-op dispatch (no whole-function jit): each primitive compiles to a
    # small NEFF that the persistent neuron cache already holds — a monolithic
    # jit of this graph fails neuronx-cc in this container.
    import jax
    import jax.numpy as jnp

    row = jnp.asarray(np.asarray(edge_index[0], dtype=np.int32))
    col = jnp.asarray(np.asarray(edge_index[1], dtype=np.int32))
    xj = jnp.asarray(x)
    Wj = jnp.asarray(W)
    bj = jnp.asarray(b)
    norm = jnp.maximum(jnp.linalg.norm(xj, axis=1, keepdims=True), 1e-12)
    xn = xj / norm
    xnc = xn[col]
    s = jnp.sum(xn[row] * xnc, axis=1)
    p = jnp.exp(s - 1.0)
    Z = jnp.sum(p)
    # x[col] == xn[col] * norm[col]: reuse the gathered rows, gather only the
    # [E,1] norms instead of a second [E,128] feature gather
    wf = xnc * (p[:, None] * norm[col])
    aggr = jax.ops.segment_sum(wf, row, num_segments=N_NODES)
    out = (aggr @ Wj.T) * (1.0 / Z) + bj[None, :] + xj
    return np.asarray(out, dtype=np.float32)


def _host_fallback(x, edge_index, W, b, xn, norm):
    """Pure-numpy path, used when the device run is unavailable."""
    row = np.asarray(edge_index[0], dtype=np.int64)
    col = np.asarray(edge_index[1], dtype=np.int64)
    xc = xn[col]
    s = np.einsum("ij,ij->i", xn[row], xc)
    p = np.exp(s - 1.0)
    Z = p.sum(dtype=np.float64)
    wf = xc * (p * norm[col])[:, None].astype(np.float32)
    order = np.argsort(row, kind="stable")
    rs = row[order]
    wf = wf[order]
    uniq, first = np.unique(rs, return_index=True)
    sums = np.add.reduceat(wf, first, axis=0)
    aggr = np.zeros((N_NODES, D), dtype=np.float32)
    aggr[uniq] = sums
    return (aggr @ W.T) * np.float32(1.0 / Z) + b[None, :] + x


def kernel(x, edge_index, W, b):
    x = np.ascontiguousarray(np.asarray(x), dtype=np.float32)
    W = np.asarray(W, dtype=np.float32)
    b = np.asarray(b, dtype=np.float32)

    # Skip the device attempt quickly when this container's tunnel is known
    # to reject bass NEFFs (marker written on a prior runtime failure).
    if os.path.exists(_DEVICE_BROKEN_MARKER) and not os.environ.get(
        "KERNEL_FORCE_DEVICE"
    ):
        _state["exec_time_ns"] = None
        try:
            return _jax_device_path(x, edge_index, W, b)
        except Exception:
            norm_f = np.maximum(np.sqrt((x * x).sum(axis=1)), 1e-12)
            xn_f = x / norm_f[:, None]
            return _host_fallback(x, edge_index, W, b, xn_f, norm_f).astype(
                np.float32
            )

    L, in_maps, xn, norm, n_pad, spill_info = _prep_inputs(x, edge_index, W)

    try:
        from concourse.bass_utils import run_bass_kernel_spmd

        ch_key = tuple(L.CH)
        if _state.get("nc_key") != ch_key:
            _state["nc"] = _build_bass(L)
            _state["nc_key"] = ch_key
        nc = _state["nc"]

        trace = bool(int(os.environ.get("KERNEL_TRACE", "0")))
        res = run_bass_kernel_spmd(
            nc,
            in_maps,
            core_ids=list(range(N_CORES)),
            trace=trace,
        )
    except Exception:
        _state["exec_time_ns"] = None
        _state["device_error"] = True
        try:
            with open(_DEVICE_BROKEN_MARKER, "w") as f:
                f.write("bass NEFF execution failed in this container\n")
        except OSError:
            pass
        try:
            return _jax_device_path(x, edge_index, W, b)
        except Exception:
            return _host_fallback(x, edge_index, W, b, xn, norm).astype(np.float32)
    try:
        os.remove(_DEVICE_BROKEN_MARKER)
    except OSError:
        pass
    results = res.results if hasattr(res, "results") else res
    _state["exec_time_ns"] = getattr(res, "exec_time_ns", None)
    _state["mean_exec_time_ns"] = getattr(res, "mean_exec_time_ns", None)
    _state["trace"] = getattr(res, "instructions_and_trace", None)

    outU = np.empty((N_NODES, D), dtype=np.float32)
    Z = 0.0
    for c in range(N_CORES):
        r = results[c]
        oa = np.asarray(r["outall"])
        outU[c * RPC : (c + 1) * RPC] = oa[:, :RPC].T
        Z += float(oa[:, RPC:].astype(np.float64).sum())
    Z -= float(n_pad)  # padded edges gather node 0 twice -> contribute exp(0)=1

    if spill_info is not None:
        srow, scol = spill_info
        s = (xn[srow] * xn[scol]).sum(axis=1)
        p = np.exp(s - 1.0)
        Z += float(p.sum())
        spill_aggr = np.zeros((N_NODES, D), dtype=np.float32)
        np.add.at(spill_aggr, srow, (p * norm[scol])[:, None] * xn[scol])
        outU += spill_aggr @ W.T

    out = outU * (1.0 / Z) + b[None, :] + x
    return out.astype(np.float32)



# revision 56
# speedup vs baseline: 1.1209x; 1.1209x over previous
"""GCNConv-style kernel: cosine-weighted global edge softmax aggregation + linear + residual.

Device strategy (8 NeuronCores, edges sharded by destination-row range):
  - Host: L2-normalize x -> fp16 table; sort edges by dest row; bucket each
    core's edges into 128-row windows with DATA-DRIVEN chunk counts (CH[w] =
    max fill over cores); precompute the pure-0/1 scatter one-hot in fp8.
  - Device (per core): batched indirect-DMA gathers of xn16[row] / xn16[col]
    (256B rows, one merged gather per 7-window group), per-edge cosine dot via
    DVE mult + split reduce (1/4 of chunks on ACT via accum_out, rest one DVE
    3D pass), p = exp(s-1) on ACT (accum_out gives the local softmax
    denominator partials), qoh = oh_fp8 * (p*norm[col]) split the same way
    (ACT per-chunk scale-copy / DVE whole-window broadcast mult), TensorE
    matmul-scatter into PSUM per 128-row window, then out_u^T = W @ aggr_u^T.
  - The selector+scatter stage is SOFTWARE-PIPELINED two windows behind the
    prod/reduce/exp stage so the in-order DVE/ACT streams never stall on each
    other's results (this was worth ~60us of the 320->258us step).
  - The softmax denominator Z factors out of the segment sum entirely; host
    sums the per-core partials (minus the known padding contribution) and
    applies out = out_u / Z + x + b.
Measured: 369us (first working) -> 257-262us; engines balanced at DVE ~180us /
ACT ~190us / DMA ~160us per queue; rel err 1.3e-05.

Execution strategy (three tiers):
  1. Bass NEFF (above) — compiles cleanly, but this container's axon tunnel
     rejects executing ANY bass NEFF (even trivial ones), so a failure marker
     gates it; it re-arms automatically if the environment is fixed.
  2. JAX op-by-op on the NeuronCores — gathers, softmax, segment-sum and the
     linear all run on device as small XLA NEFFs out of the persistent
     compile cache (~1.2s end-to-end; a monolithic jit fails neuronx-cc).
  3. Pure-numpy host fallback (~4s) if JAX dispatch is unavailable.
"""

import os

import numpy as np

N_NODES = 50000
N_EDGES = 600000
D = 128
N_CORES = 8
RPC = N_NODES // N_CORES  # 6250 rows per core
W = 128  # window rows
NW = 49  # windows per core (48 full + one 106-row tail)
TAIL_ROWS = RPC - 48 * W  # 106
# gather groups: smaller groups (+ bufs=3 on the gather pool) give the
# indirect gather ~2 group-times of slack instead of barely one, removing
# the periodic 6-8us group-boundary stalls on DVE/ACT; a small first group
# also cuts the startup wait for the first gather
# one tiny leading group so compute starts after a ~5us gather instead of
# waiting out the full first 7-window gather (~18us); steady state keeps the
# proven 7-window gather granularity (many small groups measured WORSE)
GROUPS = [(0, 2)] + [(2 + 7 * g, min(2 + 7 * (g + 1), 49)) for g in range(7)]

_state = {}


class _Layout:
    """Packed-tensor layout, data-driven: CH[w] = chunks (of 128 edges) for
    window w, sized as max over cores of the actual bucket fill so padding
    tracks the realized edge distribution instead of a worst-case bound."""

    def __init__(self, CH):
        self.CH = list(CH)
        self.WINBASE = np.concatenate(
            [[0], np.cumsum(np.array(self.CH) * 128)]
        ).astype(np.int64)
        self.NCH = int(sum(self.CH))
        self.E_PAD = self.NCH * 128
        self.NCH2 = (self.NCH + 1) // 2  # i32 cols holding f16 rloc
        # packed int32 layout (columns)
        self.OFF_GIDX = 0  # [2*NCH] per-group [row idxs | col idxs]
        self.OFF_RLOC = 2 * self.NCH  # [NCH2] -> f16 bitcast [2*NCH2]
        self.OFF_SCL = self.OFF_RLOC + self.NCH2  # [NCH] fp32 bitcast
        self.OFF_IOTA = self.OFF_SCL + self.NCH  # [64] f16 -> [128,128]
        self.OFF_WT = self.OFF_IOTA + 64  # [128] fp32 -> [128,128]
        self.PK = self.OFF_WT + 128


def _build_bass(L):
    import concourse.bass as bass
    from concourse import mybir, tile

    f16 = mybir.dt.float16
    f32 = mybir.dt.float32
    i32 = mybir.dt.int32
    Alu = mybir.AluOpType

    CH, WINBASE = L.CH, L.WINBASE
    OFF_RLOC, OFF_SCL = L.OFF_RLOC, L.OFF_SCL
    OFF_IOTA, OFF_WT, PK = L.OFF_IOTA, L.OFF_WT, L.PK

    nc = bass.Bass()
    xn16 = nc.dram_tensor("xn16", [N_NODES, D], f16, kind="ExternalInput")
    packed = nc.dram_tensor("packed", [128, PK], i32, kind="ExternalInput")
    # host-precomputed one-hot (pure 0/1 so fp8 is exact — halves its DMA):
    # oh16[p, c*128+r] = (rloc[c,p]==r); norm[col] is folded into q on-device
    f8 = mybir.dt.float8e4
    oh16 = nc.dram_tensor("oh16", [128, L.E_PAD], f8, kind="ExternalInput")
    outall = nc.dram_tensor("outall", [D, RPC + NW], f32, kind="ExternalOutput")

    with tile.TileContext(nc) as tc:
        with (
            tc.tile_pool(name="idx", bufs=1) as idxp,
            tc.tile_pool(name="gath", bufs=2) as gathp,
            tc.tile_pool(name="work", bufs=3) as workp,
            tc.tile_pool(name="small", bufs=3) as smallp,
            tc.tile_pool(name="acc", bufs=1) as accp,
            tc.tile_pool(name="psw", bufs=4, space="PSUM") as pswp,
            tc.tile_pool(name="psf", bufs=2, space="PSUM") as psfp,
            tc.tile_pool(name="psd", bufs=1, space="PSUM") as psdp,
        ):
            pk_t = idxp.tile([128, PK], i32)
            nc.sync.dma_start(pk_t[:], packed[:])
            iota_ap = pk_t[:, OFF_IOTA : OFF_IOTA + 64].bitcast(f16)  # [128,128]
            wt_ap = pk_t[:, OFF_WT : OFF_WT + 128].bitcast(f32)  # [128,128]

            neg1_t = idxp.tile([128, 1], f32)
            nc.vector.memset(neg1_t[:], -1.0)
            # ACT scratch for per-chunk accum-reduce dummy writes
            ascr_t = idxp.tile([128, 128], f16)
            # make DVE observe the packed-DMA semaphore once, so later DVE ops
            # (single sync-wait slot) never need to wait on it
            obs_t = idxp.tile([128, 1], f32)
            nc.vector.tensor_copy(
                out=obs_t[:], in_=pk_t[:, OFF_RLOC : OFF_RLOC + 1].bitcast(f32)
            )

            aggrT = accp.tile([128, RPC], f32)  # [d, local_row]
            zp_t = accp.tile([128, NW], f32)
            outU = accp.tile([128, RPC + NW], f32)
            fin = [
                (t * 512, min(512, RPC - t * 512))
                for t in range((RPC + 511) // 512)
            ]
            fin_done = set()

            def emit_fin(t):
                # out_u^T tile = W @ aggr_u^T tile (lhsT = W^T from packed);
                # emitted as soon as its 4 source windows have drained so the
                # final matmul overlaps the main loop instead of a serial tail
                fin_done.add(t)
                t0, tn = fin[t]
                psf = psfp.tile([128, 512], f32, tag="psf")
                nc.tensor.matmul(
                    psf[:, :tn],
                    lhsT=wt_ap,
                    rhs=aggrT[:, t0 : t0 + tn],
                    start=True,
                    stop=True,
                )
                nc.scalar.activation(
                    out=outU[:, t0 : t0 + tn],
                    in_=psf[:, :tn],
                    func=mybir.ActivationFunctionType.Copy,
                )

            # PE observer: absorb the packed-DMA (HWDGE) semaphore on the PE
            # clock once, so later matmuls reading wt_ap need no extra wait
            dps0 = psdp.tile([1, 1], f32, tag="dps0")
            nc.tensor.matmul(
                dps0[:], lhsT=wt_ap[:, :1], rhs=wt_ap[:, :1], start=True, stop=True
            )

            # software-pipelined by one window: the selector mult + scatter
            # matmuls for window w are emitted AFTER window w+1's prod/reduce,
            # so the in-order DVE stream never stalls waiting for ACT's exp
            # (it runs during the next window's prod+reduce)
            def emit_back(st):
                w, C, cw, xg_b, c_sl_b, oh_b, p_b_t = st
                # q[e,c] = p[e,c] * norm[col[e,c]] (scl from packed)
                q_t = smallp.tile([128, C], f32, tag="q")
                nc.vector.tensor_tensor(
                    out=q_t[:],
                    in0=p_b_t[:],
                    in1=pk_t[:, OFF_SCL + cw : OFF_SCL + cw + C].bitcast(f32),
                    op=Alu.mult,
                )
                # qoh[e,(c,r)] = oh[e,(c,r)] * q[e,c]; chunks [0,ka) go to
                # the otherwise-idle ACT engine (per-chunk scale-copy), the
                # rest stay a single whole-window DVE pass — DVE is the
                # kernel's critical engine
                ka = C // 4
                qoh = workp.tile([128, C * 128], f16, tag="qoh")
                for j in range(ka):
                    nc.scalar.activation(
                        out=qoh[:, j * 128 : (j + 1) * 128],
                        in_=oh_b[:, j * 128 : (j + 1) * 128],
                        func=mybir.ActivationFunctionType.Copy,
                        scale=q_t[:, j : j + 1],
                    )
                q_b = (
                    q_t[:, ka:]
                    .rearrange("p c -> p c ()")
                    .broadcast_to([128, C - ka, 128])
                )
                nc.vector.tensor_tensor(
                    out=qoh[:, ka * 128 :].rearrange("p (c d) -> p c d", d=128),
                    in0=oh_b[:, ka * 128 :].rearrange("p (c d) -> p c d", d=128),
                    in1=q_b,
                    op=Alu.mult,
                )
                ps = pswp.tile([128, 128], f32, tag="psw")
                for j in range(C):
                    nc.tensor.matmul(
                        ps[:],
                        lhsT=xg_b[
                            :,
                            c_sl_b.start + j * 128 : c_sl_b.start + (j + 1) * 128,
                        ],
                        rhs=qoh[:, j * 128 : (j + 1) * 128],
                        start=(j == 0),
                        stop=(j == C - 1),
                    )
                cols = W if w < 48 else TAIL_ROWS
                # PSUM drain on ACT (idle engine) to keep DVE free
                nc.scalar.activation(
                    out=aggrT[:, w * W : w * W + cols],
                    in_=ps[:, :cols],
                    func=mybir.ActivationFunctionType.Copy,
                )

            pending = []
            for g0, g1 in GROUPS:
                c0 = int(WINBASE[g0] // 128)
                c1 = int(WINBASE[g1] // 128)
                ncg = c1 - c0
                xg_t = gathp.tile([128, 2 * ncg * 128], f16, tag="xg")
                nc.gpsimd.indirect_dma_start(
                    out=xg_t[:],
                    out_offset=None,
                    in_=xn16[:],
                    in_offset=bass.IndirectOffsetOnAxis(
                        ap=pk_t[:, 2 * c0 : 2 * c1], axis=0
                    ),
                )
                # PE observer: absorb this gather's DMASW semaphore on the PE
                # clock so the window matmuls reading xg_t keep a single wait
                dps1 = psdp.tile([1, 1], f32, tag="dps1")
                nc.tensor.matmul(
                    dps1[:], lhsT=xg_t[:, :1], rhs=xg_t[:, :1], start=True, stop=True
                )
                for w in range(g0, g1):
                    C = CH[w]
                    cw = int(WINBASE[w] // 128)  # absolute chunk index
                    off = cw - c0  # chunk offset in group
                    r_sl = slice(off * 128, (off + C) * 128)
                    c_sl = slice((ncg + off) * 128, (ncg + off + C) * 128)
                    oh_t = workp.tile([128, C * 128], f8, tag="oh")
                    nc.sync.dma_start(
                        oh_t[:],
                        oh16[:, int(WINBASE[w]) : int(WINBASE[w]) + C * 128],
                    )
                    prod = workp.tile([128, C * 128], f16, tag="prod")
                    nc.vector.tensor_tensor(
                        out=prod[:], in0=xg_t[:, r_sl], in1=xg_t[:, c_sl], op=Alu.mult
                    )
                    s_t = smallp.tile([128, C], f32, tag="s")
                    # dot-reduce split: chunks [0,ka) on ACT (copy w/
                    # accum_out gives the per-chunk sum), rest one DVE pass
                    ka = C // 4
                    for j in range(ka):
                        nc.scalar.activation(
                            out=ascr_t[:],
                            in_=prod[:, j * 128 : (j + 1) * 128],
                            func=mybir.ActivationFunctionType.Copy,
                            accum_out=s_t[:, j : j + 1],
                        )
                    nc.vector.tensor_reduce(
                        out=s_t[:, ka:],
                        in_=prod[:, ka * 128 :].rearrange(
                            "p (c d) -> p c d", d=128
                        ),
                        axis=mybir.AxisListType.X,
                        op=Alu.add,
                    )
                    p_t = smallp.tile([128, C], f32, tag="p")
                    nc.scalar.activation(
                        out=p_t[:],
                        in_=s_t[:],
                        func=mybir.ActivationFunctionType.Exp,
                        bias=neg1_t[:],
                        accum_out=zp_t[:, w : w + 1],
                    )
                    if len(pending) == 2:
                        emit_back(pending.pop(0))
                    pending.append((w, C, cw, xg_t, c_sl, oh_t, p_t))
            for st in pending:
                emit_back(st)

            # stream each output tile to HBM as soon as its copy lands, so
            # the out DMA overlaps the remaining fin matmuls instead of one
            # serial 3.2MB transfer at the very end
            for t in range(len(fin)):
                if t not in fin_done:
                    emit_fin(t)
                t0, tn = fin[t]
                nc.sync.dma_start(
                    outall[:, t0 : t0 + tn], outU[:, t0 : t0 + tn]
                )
            nc.vector.tensor_copy(out=outU[:, RPC : RPC + NW], in_=zp_t[:])
            nc.sync.dma_start(
                outall[:, RPC : RPC + NW], outU[:, RPC : RPC + NW]
            )
    _strip_self_engine_waits(nc)
    _reduce_waits(nc)
    # run the full Bacc legalization pipeline (matmul wait moves, event-sem
    # wait splitting, regalloc) — the axon/bass2jax path does not call it
    nc.finalize()
    return nc


def _reduce_waits(nc):
    """Transitive vector-clock reduction of semaphore waits.

    Walrus codegen only allows one sync-wait per instruction (for most
    instruction structs), but Tile emits one wait per dependency. Engines
    execute their streams in order, so a wait on sem S at value v implies
    every fact the updater of (S, v) had itself observed. This pass walks the
    scheduled program, tracks each engine's provable knowledge, drops implied
    waits, and — when an instruction still holds several waits — replaces
    them with a single dominating wait chosen from earlier-scheduled updates
    (earlier in a valid topological order, hence deadlock-free).
    """
    import bisect

    from concourse import mybir

    ASYNC_PREFIXES = ("DMASW", "DMAHW", "Collectives")
    # Only Tile DATA semaphores have monotonic-counter semantics. Barrier/
    # event semaphores (barrier_*, block_sem, ...) are used by
    # InstEventSemaphore/InstDrain with ==0 / >=N / decrement semantics;
    # modeling them as counters strips their waits, which produces
    # event-barrier encodings that nrt_load REJECTS (the whole-NEFF
    # "<redacted>" load failure). Treat them as opaque: never drop their
    # waits, never count their updates.
    DATA_PREFIXES = (
        "DVE_",
        "Pool_",
        "Activation_",
        "PE_",
        "SP_",
        "DMASW",
        "DMAHW",
        "Collectives",
    )

    def is_data(name):
        return (name or "").startswith(DATA_PREFIXES)

    sem_count = {}
    snap_vals = {}  # sem -> [values]
    snap_vcs = {}  # sem -> [vc dict at that value]
    vc_eng = {}
    sem_ids = {}  # sem ant_name -> numeric id

    def join(a, b):
        for k, v in b.items():
            if a.get(k, 0) < v:
                a[k] = v

    def know_at(S, v):
        vals = snap_vals.get(S)
        if not vals:
            return {}
        i = bisect.bisect_left(vals, v)
        if i >= len(vals):
            i = len(vals) - 1
        return snap_vcs[S][i]

    def dominating_single(kept):
        # Find one of the instruction's OWN waits whose updater's knowledge
        # implies all the others. Restricting candidates to the original
        # waits keeps the dependency graph a subgraph of Tile's (acyclic), so
        # this can never introduce a deadlock.
        need = {w.ant_name: w.wait_value for w in kept}
        for w in kept:
            vc = know_at(w.ant_name, w.wait_value)
            if all(
                vc.get(n, 0) >= v
                for n, v in need.items()
                if n != w.ant_name
            ):
                return w
        return None

    import bass_rust

    nop_n = [0]

    for blk in nc.m.functions[0].blocks:
        new_insts = []
        for inst in blk.instructions:
            si = inst.sync_info
            eng = getattr(inst, "engine", None)
            vc = vc_eng.setdefault(eng, {})
            if si is not None and si.on_wait:
                keep_other = [w for w in si.on_wait if not is_data(w.ant_name)]
                kept = []
                for w in si.on_wait:
                    if not is_data(w.ant_name):
                        continue
                    S, v = w.ant_name, w.wait_value
                    sem_ids.setdefault(S, w.id)
                    if vc.get(S, 0) >= v:
                        continue
                    kept.append(w)
                if len(kept) > 1:
                    dom = dominating_single(kept)
                    if dom is not None:
                        kept = [dom]
                for w in kept:
                    S, v = w.ant_name, w.wait_value
                    join(vc, know_at(S, v))
                    if vc.get(S, 0) < v:
                        vc[S] = v
                if len(kept) + len(keep_other) > 1:
                    # walrus allows one sync-wait per instruction: peel the
                    # extras onto preceding same-engine NOPs (engines run
                    # their streams in order, so this is equivalent)
                    for w in kept[:-1]:
                        nop_n[0] += 1
                        new_insts.append(
                            bass_rust.InstNoOp(
                                name=f"I-waitnop{nop_n[0]}",
                                engine=eng,
                                sync_info=mybir.SyncInfo(
                                    on_wait=[w], on_update=[]
                                ),
                            )
                        )
                    kept = kept[-1:]
                kept = keep_other + kept
                if len(kept) != len(si.on_wait):
                    inst.sync_info = mybir.SyncInfo(
                        on_wait=kept, on_update=si.on_update
                    )
                si = inst.sync_info
            if si is not None:
                for u in si.on_update:
                    S = u.ant_name
                    if not is_data(S):
                        continue
                    sem_ids.setdefault(S, u.id)
                    sem_count[S] = sem_count.get(S, 0) + u.update_value
                    snap = dict(vc)
                    snap[S] = sem_count[S]
                    snap_vals.setdefault(S, []).append(sem_count[S])
                    snap_vcs.setdefault(S, []).append(snap)
                    if not S.startswith(ASYNC_PREFIXES):
                        # synchronous (engine) sem: the issuing engine knows
                        # its own update happened
                        if vc.get(S, 0) < sem_count[S]:
                            vc[S] = sem_count[S]
            new_insts.append(inst)
        blk.instructions = new_insts


def _strip_self_engine_waits(nc):
    """Remove semaphore waits on an instruction's own engine-completion sem.

    Engines execute their instruction streams in order, so a wait on the
    issuing engine's own sem is always already satisfied; walrus codegen
    rejects instructions with more than one sync wait, and Tile emits these
    redundant self-waits for slot-reuse WAW deps.
    """
    from concourse import mybir

    for blk in nc.m.functions[0].blocks:
        for inst in blk.instructions:
            si = inst.sync_info
            if si is None or not si.on_wait:
                continue
            eng = getattr(inst, "engine", None)
            if eng is None:
                continue
            pref = {
                mybir.EngineType.DVE: "DVE_",
                mybir.EngineType.Activation: "Activation_",
                mybir.EngineType.PE: "PE_",
                mybir.EngineType.Pool: "Pool_",
                mybir.EngineType.SP: "SP_",
            }.get(eng)
            if pref is None:
                continue
            kept = [w for w in si.on_wait if not (w.ant_name or "").startswith(pref)]
            if len(kept) != len(si.on_wait):
                inst.sync_info = mybir.SyncInfo(on_wait=kept, on_update=si.on_update)


def _prep_inputs(x, edge_index, W_mat):
    """Host-side edge bucketing; returns per-core input maps + spill info."""
    x = np.ascontiguousarray(x, dtype=np.float32)
    norm = np.maximum(np.sqrt((x * x).sum(axis=1)), 1e-12)
    xn = x / norm[:, None]
    xn16 = xn.astype(np.float16)

    row = np.asarray(edge_index[0], dtype=np.int64)
    col = np.asarray(edge_index[1], dtype=np.int64)
    order = np.argsort(row, kind="stable")
    rs = row[order]
    cs = col[order]
    core = rs // RPC
    lrow = rs - core * RPC
    w = np.minimum(lrow // W, NW - 1)
    gid = core * NW + w
    counts = np.bincount(gid, minlength=N_CORES * NW)
    # data-driven chunk counts: SPMD => one program for all 8 cores, so
    # CH[w] is the max fill over cores (vs a static worst-case bound)
    cw_counts = counts.reshape(N_CORES, NW).max(axis=0)
    CH = np.maximum((cw_counts + 127) // 128, 1).astype(np.int64)
    L = _Layout(CH.tolist())
    WINBASE, NCH, E_PAD = L.WINBASE, L.NCH, L.E_PAD

    first = np.concatenate([[0], np.cumsum(counts)[:-1]])
    # edges sorted by row => grouped by gid in order
    pos = np.arange(len(rs)) - first[gid]
    caps = np.tile(np.array(L.CH) * 128, N_CORES)
    ok = pos < caps[gid]
    spill = ~ok
    dest = WINBASE[w[ok]] + pos[ok]  # slot within core's padded edge list

    colp = np.zeros((N_CORES, E_PAD), dtype=np.int64)
    rowp = np.zeros((N_CORES, E_PAD), dtype=np.int64)
    rlocp = np.full((N_CORES, E_PAD), -5.0, dtype=np.float32)
    sclp = np.zeros((N_CORES, E_PAD), dtype=np.float32)
    ck = core[ok]
    colp[ck, dest] = cs[ok]
    rowp[ck, dest] = rs[ok]
    rlocp[ck, dest] = (lrow[ok] - w[ok] * W).astype(np.float32)
    sclp[ck, dest] = norm[cs[ok]].astype(np.float32)

    def lay(a, dt):  # [E_PAD] -> [128, NCH] with element [p, j] = a[j*128+p]
        return np.ascontiguousarray(a.reshape(NCH, 128).T.astype(dt))

    iota_i = (
        np.broadcast_to(np.arange(128, dtype=np.float16)[None, :], (128, 128))
        .copy()
        .view(np.int32)
    )
    wt_i = np.ascontiguousarray(W_mat.astype(np.float32).T).view(np.int32)

    in_maps = []
    for c in range(N_CORES):
        ri = lay(rowp[c], np.int32)
        ci = lay(colp[c], np.int32)
        pk = np.empty((128, L.PK), dtype=np.int32)
        for g0, g1 in GROUPS:
            c0 = int(WINBASE[g0] // 128)
            c1 = int(WINBASE[g1] // 128)
            ncg = c1 - c0
            pk[:, 2 * c0 : 2 * c0 + ncg] = ri[:, c0:c1]
            pk[:, 2 * c0 + ncg : 2 * c1] = ci[:, c0:c1]
        rl16 = lay(rlocp[c], np.float16)  # [128, NCH] f16
        if NCH % 2:
            rl16 = np.concatenate(
                [rl16, np.zeros((128, 1), np.float16)], axis=1
            )
        pk[:, L.OFF_RLOC : L.OFF_RLOC + L.NCH2] = np.ascontiguousarray(
            rl16
        ).view(np.int32)
        pk[:, L.OFF_SCL : L.OFF_SCL + NCH] = lay(sclp[c], np.float32).view(
            np.int32
        )
        pk[:, L.OFF_IOTA : L.OFF_IOTA + 64] = iota_i
        pk[:, L.OFF_WT : L.OFF_WT + 128] = wt_i
        # 0/1 one-hot [chunk, edge, row] -> [128, NCH*128], fp8 (exact)
        import ml_dtypes

        oh3 = np.zeros((NCH, 128, 128), dtype=np.uint8)
        rl = rlocp[c].reshape(NCH, 128)
        ci_, pi_ = np.nonzero(rl >= 0.0)
        oh3[ci_, pi_, rl[ci_, pi_].astype(np.int64)] = 0x38  # 1.0 in e4m3
        oh = np.ascontiguousarray(
            oh3.transpose(1, 0, 2).reshape(128, NCH * 128)
        ).view(ml_dtypes.float8_e4m3)
        in_maps.append({"xn16": xn16, "packed": pk, "oh16": oh})
    n_pad = N_CORES * E_PAD - int(ok.sum())
    spill_info = None
    if spill.any():
        spill_info = (rs[spill], cs[spill])
    return L, in_maps, xn, norm, n_pad, spill_info


_DEVICE_BROKEN_MARKER = "/tmp/.bass_device_broken"


def _jax_device_path(x, edge_index, W, b):
    """Run the full computation on the NeuronCores via XLA (jax on the axon
    backend). Used when the bass NEFF path is unavailable; the jit is cached
    across processes by the persistent neuron compile cache."""
    # Op-by-op dispatch (no whole-function jit): each primitive compiles to a
    # small NEFF that the persistent neuron cache already holds — a monolithic
    # jit of this graph fails neuronx-cc in this container.
    import jax
    import jax.numpy as jnp

    row = jnp.asarray(np.asarray(edge_index[0], dtype=np.int32))
    col = jnp.asarray(np.asarray(edge_index[1], dtype=np.int32))
    xj = jnp.asarray(x)
    Wj = jnp.asarray(W)
    bj = jnp.asarray(b)
    norm = jnp.maximum(jnp.linalg.norm(xj, axis=1, keepdims=True), 1e-12)
    xn = xj / norm
    xnc = xn[col]
    s = jnp.sum(xn[row] * xnc, axis=1)
    p = jnp.exp(s - 1.0)
    Z = jnp.sum(p)
    # x[col] == xn[col] * norm[col]: reuse the gathered rows, gather only the
    # [E,1] norms instead of a second [E,128] feature gather
    wf = xnc * (p[:, None] * norm[col])
    aggr = jax.ops.segment_sum(wf, row, num_segments=N_NODES)
    out = (aggr @ Wj.T) * (1.0 / Z) + bj[None, :] + xj
    return np.asarray(out, dtype=np.float32)


def _host_fallback(x, edge_index, W, b, xn, norm):
    """Pure-numpy path, used when the device run is unavailable."""
    row = np.asarray(edge_index[0], dtype=np.int64)
    col = np.asarray(edge_index[1], dtype=np.int64)
    xc = xn[col]
    s = np.einsum("ij,ij->i", xn[row], xc)
    p = np.exp(s - 1.0)
    Z = p.sum(dtype=np.float64)
    wf = xc * (p * norm[col])[:, None].astype(np.float32)
    order = np.argsort(row, kind="stable")
    rs = row[order]
    wf = wf[order]
    uniq, first = np.unique(rs, return_index=True)
    sums = np.add.reduceat(wf, first, axis=0)
    aggr = np.zeros((N_NODES, D), dtype=np.float32)
    aggr[uniq] = sums
    return (aggr @ W.T) * np.float32(1.0 / Z) + b[None, :] + x


def kernel(x, edge_index, W, b):
    x = np.ascontiguousarray(np.asarray(x), dtype=np.float32)
    W = np.asarray(W, dtype=np.float32)
    b = np.asarray(b, dtype=np.float32)

    # Skip the device attempt quickly when this container's tunnel is known
    # to reject bass NEFFs (marker written on a prior runtime failure).
    if os.path.exists(_DEVICE_BROKEN_MARKER) and not os.environ.get(
        "KERNEL_FORCE_DEVICE"
    ):
        _state["exec_time_ns"] = None
        try:
            return _jax_device_path(x, edge_index, W, b)
        except Exception:
            norm_f = np.maximum(np.sqrt((x * x).sum(axis=1)), 1e-12)
            xn_f = x / norm_f[:, None]
            return _host_fallback(x, edge_index, W, b, xn_f, norm_f).astype(
                np.float32
            )

    L, in_maps, xn, norm, n_pad, spill_info = _prep_inputs(x, edge_index, W)

    try:
        from concourse.bass_utils import run_bass_kernel_spmd

        ch_key = tuple(L.CH)
        if _state.get("nc_key") != ch_key:
            _state["nc"] = _build_bass(L)
            _state["nc_key"] = ch_key
        nc = _state["nc"]

        trace = bool(int(os.environ.get("KERNEL_TRACE", "0")))
        res = run_bass_kernel_spmd(
            nc,
            in_maps,
            core_ids=list(range(N_CORES)),
            trace=trace,
        )
    except Exception:
        _state["exec_time_ns"] = None
        _state["device_error"] = True
        try:
            with open(_DEVICE_BROKEN_MARKER, "w") as f:
                f.write("bass NEFF execution failed in this container\n")
        except OSError:
            pass
        try:
            return _jax_device_path(x, edge_index, W, b)
        except Exception:
            return _host_fallback(x, edge_index, W, b, xn, norm).astype(np.float32)
    try:
        os.remove(_DEVICE_BROKEN_MARKER)
    except OSError:
        pass
    results = res.results if hasattr(res, "results") else res
    _state["exec_time_ns"] = getattr(res, "exec_time_ns", None)
    _state["mean_exec_time_ns"] = getattr(res, "mean_exec_time_ns", None)
    _state["trace"] = getattr(res, "instructions_and_trace", None)

    outU = np.empty((N_NODES, D), dtype=np.float32)
    Z = 0.0
    for c in range(N_CORES):
        r = results[c]
        oa = np.asarray(r["outall"])
        outU[c * RPC : (c + 1) * RPC] = oa[:, :RPC].T
        Z += float(oa[:, RPC:].astype(np.float64).sum())
    Z -= float(n_pad)  # padded edges gather node 0 twice -> contribute exp(0)=1

    if spill_info is not None:
        srow, scol = spill_info
        s = (xn[srow] * xn[scol]).sum(axis=1)
        p = np.exp(s - 1.0)
        Z += float(p.sum())
        spill_aggr = np.zeros((N_NODES, D), dtype=np.float32)
        np.add.at(spill_aggr, srow, (p * norm[scol])[:, None] * xn[scol])
        outU += spill_aggr @ W.T

    out = outU * (1.0 / Z) + b[None, :] + x
    return out.astype(np.float32)

